# revision 12
# baseline (speedup 1.0000x reference)
"""ChatGLM2 attention block on 8 Trainium2 NeuronCores (Bass/Tile).

Sharding: tensor-parallel across heads for QKV+attention; each core c owns
Q heads 4c..4c+3 (512 dims). K/V projection is split 4-ways inside each KV
group: core with group-rank r computes a 64-col [K 32 | V 32] slice, and a
small intra-group AllGather (128KB in) rebuilds the full K/V per batch.
Dense is sharded 2x4 (token-half x output-quarter): the per-core context is
redistributed with an AllToAll (stride-0-broadcast input, 2MB staged)
instead of a 16MB AllGather, and each core computes out[1024 od, 1024 tok]
against its w_dense column slice.

DMA triggers are the scarce resource (~0.7us dispatch each, serialized per
engine sequencer): they are batched into few large transfers (packed
consts, one DMA per weight group, packed rope planes, one mega-AP DMA for
the whole gathered-context tile) and spread across the gpsimd / vector /
scalar queues so the sync queue never backs up in front of a collective
trigger or a proj input load.

All matmuls run fp16 (1 cycle/row). Softmax skips the row-max (scores are
~1e-2); the denominator is a ones-matrix matmul accumulated alongside the
AV matmul; causal masking is a DVE multiply with a 128x128 triangular tile
on only the diagonal 128 columns of each straddling k-tile. The attention
inner loop is software-pipelined with lookahead 2 so the PE never waits on
the exp->mask chain.
"""

import math
import sys
import types

import numpy as np

# ---------------------------------------------------------------- constants
B, S, H = 2, 1024, 4096
NH, G, HD = 32, 2, 128
ROT = 64
N_CORES = 8
TOK = B * S                      # 2048
HPC = NH // N_CORES              # 4 Q heads per core
DPC = HPC * HD                   # 512 Q dims per core
NDB = 5                          # per-core qkv dim blocks: 4x Q(128) + KV(64)
WQC = DPC + 64                   # 576 qkv cols per core
TB = 4                           # token blocks of 512
QB = 2                           # q blocks of 512 per batch
ODPC = H // 4                    # 1024 output dims per core (dense quarter)
SCALE = 1.0 / math.sqrt(HD)


def _install_ntff_hook():
    """The agent image's antenv lacks axon_hooks; shim it so
    run_bass_kernel_spmd(trace=True) can profile via NTFF."""
    if "antenv.axon_hooks" in sys.modules:
        return
    import antenv  # noqa: F401

    mod = types.ModuleType("antenv.axon_hooks")
    mod._hook = None
    mod.set_axon_ntff_profile_hook = lambda h: setattr(mod, "_hook", h)
    mod.get_axon_ntff_profile_hook = lambda: mod._hook
    sys.modules["antenv.axon_hooks"] = mod
    try:
        from trn_agent_boot.trn_boot import _ntff_profile_via_ctypes

        mod._hook = _ntff_profile_via_ctypes("/opt/axon/libaxon_pjrt.so")
    except Exception:
        pass


_install_ntff_hook()

import concourse.bass as bass  # noqa: E402
import concourse.mybir as mybir  # noqa: E402
import concourse.tile as tile  # noqa: E402
from concourse import bacc  # noqa: E402
from concourse.bass_utils import run_bass_kernel_spmd  # noqa: E402

F32 = mybir.dt.float32
F16 = mybir.dt.float16
AF = mybir.ActivationFunctionType
ALU = mybir.AluOpType


# ---------------------------------------------------------------- build
def build(trace_label="k"):
    nc = bacc.Bacc("TRN2", target_bir_lowering=False, debug=False,
                   num_devices=N_CORES)

    xt_d = nc.dram_tensor("xt", [H, TOK], F16, kind="ExternalInput").ap()
    wq_d = nc.dram_tensor("wqkv", [H, WQC], F16, kind="ExternalInput").ap()
    bq_d = nc.dram_tensor("bqkv", [128, NDB], F32, kind="ExternalInput").ap()
    # packed f16 consts: [ ones(128) | tri(128) | ident(128) | perm(64c) ]
    cc_d = nc.dram_tensor("consts", [128, 448], F16, kind="ExternalInput").ap()
    # packed rope planes: ropeQ = [ra;rb] (128 rows), ropeK = [rak;rbk] (64)
    rq_d = nc.dram_tensor("ropeQ", [128, TOK], F32, kind="ExternalInput").ap()
    rk_d = nc.dram_tensor("ropeK", [64, TOK], F32, kind="ExternalInput").ap()
    wd_d = nc.dram_tensor("wd", [H, ODPC], F16, kind="ExternalInput").ap()
    out_d = nc.dram_tensor("out", [ODPC, 1024], F32, kind="ExternalOutput").ap()

    from contextlib import ExitStack

    with tile.TileContext(nc) as tc:
        with tc.tile_pool(name="consts", bufs=1) as cp, \
             tc.tile_pool(name="dram", bufs=1, space="DRAM") as dp:
            # ---- small constants (alive whole kernel)
            bias_sb = cp.tile([128, NDB], F32, tag="bias")
            cc_sb = cp.tile([128, 448], F16, tag="consts")
            nc.sync.dma_start(bias_sb[:], bq_d[:])
            nc.sync.dma_start(cc_sb[:], cc_d[:])
            oc_sb = cc_sb[:, 0:128]
            tri_sb = cc_sb[:, 128:256]
            id_sb = cc_sb[:, 256:384]
            perm_sb = cc_sb[0:64, 384:448]

            # DRAM staging for collectives
            kv_loc = [dp.tile([64, 1024], F16, tag=f"kvl{b}",
                              name=f"kvl{b}") for b in range(B)]
            kvg = [dp.tile([4, 64, 1024], F16, tag=f"kvg{b}",
                           name=f"kvg{b}") for b in range(B)]
            a2a_in = [dp.tile([8, 512, 512], F16, tag=f"a2i{b}",
                              name=f"a2i{b}") for b in range(B)]
            a2a_out = [dp.tile([8, 512, 512], F16, tag=f"a2o{b}",
                               name=f"a2o{b}") for b in range(B)]

            wq_r = wq_d.rearrange("(k p) d -> p k d", p=128)
            xt_r = xt_d.rearrange("(k p) t -> p k t", p=128)
            wd_r = wd_d.rearrange("(k p) n -> p k n", p=128)

            es = ExitStack()
            pp = es.enter_context(
                tc.tile_pool(name="ps_main", bufs=8, space="PSUM"))
            kp = es.enter_context(tc.tile_pool(name="kvp", bufs=1))
            qtp = es.enter_context(tc.tile_pool(name="qtp", bufs=1))
            xcp = es.enter_context(tc.tile_pool(name="ctxp", bufs=6))
            ep = es.enter_context(tc.tile_pool(name="exp", bufs=10))
            sp = es.enter_context(tc.tile_pool(name="att_small", bufs=3))
            wdp = es.enter_context(
                tc.tile_pool(name="wd", bufs=1, side="right"))
            es2 = ExitStack()
            wp = es2.enter_context(tc.tile_pool(name="wq", bufs=1))
            xp = es2.enter_context(tc.tile_pool(name="xt", bufs=3))
            rp = es2.enter_context(tc.tile_pool(name="rope_tmp", bufs=2))
            abp = es2.enter_context(tc.tile_pool(name="ropeab", bufs=2))
            kqp = es2.enter_context(tc.tile_pool(name="kq", bufs=2))

            # per-batch K / V(transposed) tiles
            ktile = [kp.tile([128, 1024], F16, tag=f"k{b}", name=f"k{b}")
                     for b in range(B)]
            vtile = [kp.tile([128, 1024], F16, tag=f"vt{b}", name=f"vt{b}")
                     for b in range(B)]
            vn = [kp.tile([128, 1024], F16, tag=f"vn{b}", name=f"vn{b}")
                  for b in range(B)]
            qtl = {}
            w_sb = [None] * 8
            wd_sb = []

            def load_wg(g):
                if w_sb[g] is None:
                    wg = wp.tile([128, 4, WQC], F16,
                                 tag=f"wq{g}", name=f"wq{g}")
                    nc.gpsimd.dma_start(wg[:], wq_r[:, g * 4:(g + 1) * 4, :])
                    w_sb[g] = wg

            def load_wd(i):
                wg = wdp.tile([128, 8, ODPC], F16, tag=f"wd{i}",
                              name=f"wdg{i}")
                nc.gpsimd.dma_start(wg[:], wd_r[:, i * 8:(i + 1) * 8, :])
                wd_sb.append(wg)

            def proj_block(t):
                """QKV^T projection + bias + RoPE for one 512-token block.
                Q: 4x128 dims; KV: 64 dims ([K32|V32], rank slice)."""
                b, half = t // 2, t % 2
                ps = [pp.tile([128, 512], F32, tag="bank",
                              name=f"qkvps{d}") for d in range(NDB)]
                for gg in range(4):
                    xg = xp.tile([128, 8, 512], F16, tag="xtblk")
                    nc.gpsimd.dma_start(
                        xg[:], xt_r[:, gg * 8:(gg + 1) * 8,
                                    t * 512:(t + 1) * 512])
                    for kk in range(8):
                        g, k = (gg * 8 + kk) // 4, kk % 4
                        for d in range(4):
                            nc.tensor.matmul(
                                ps[d][:],
                                w_sb[g][:, k, d * 128:(d + 1) * 128],
                                xg[:, kk, :],
                                start=(gg == 0 and kk == 0),
                                stop=(gg == 3 and kk == 7),
                            )
                        nc.tensor.matmul(
                            ps[4][0:64, :],
                            w_sb[g][:, k, 512:576],
                            xg[:, kk, :],
                            start=(gg == 0 and kk == 0),
                            stop=(gg == 3 and kk == 7),
                        )
                for h in range(HPC):
                    qt = qtp.tile([128, 512], F16, tag=f"q{h}_{t}",
                                  name=f"q{h}_{t}")
                    qtl[(h, t)] = qt
                    nc.scalar.activation(qt[:], ps[h][:], AF.Identity,
                                         bias=bias_sb[:, h:h + 1])
                kq = kqp.tile([64, 512], F16, tag="kvtile", name=f"kv{t}")
                nc.scalar.activation(kq[:], ps[4][0:64, :], AF.Identity,
                                     bias=bias_sb[0:64, 4:5])
                tsl = slice(t * 512, (t + 1) * 512)
                qab = abp.tile([128, 512], F32, tag="ropeQb")
                nc.sync.dma_start(qab[:], rq_d[:, tsl])
                kab = abp.tile([64, 512], F32, tag="ropeKb", bufs=1)
                nc.sync.dma_start(kab[:], rk_d[:, tsl])
                for e in range(HPC):
                    qt = qtl[(e, t)]
                    sw = pp.tile([128, 512], F32, tag="bank", name="swps")
                    nc.tensor.matmul(sw[0:ROT, :], perm_sb[:],
                                     qt[0:ROT, :], start=True, stop=True)
                    t1 = rp.tile([ROT, 512], F32, tag="t1")
                    nc.vector.tensor_mul(t1[:], qt[0:ROT, :], qab[0:64, :])
                    t2 = rp.tile([ROT, 512], F32, tag="t2")
                    nc.vector.tensor_mul(t2[:], sw[0:ROT, :], qab[64:128, :])
                    nc.vector.tensor_add(qt[0:ROT, :], t1[:], t2[:])
                # K slice rope (identity planes on ranks holding pass-dims)
                swk = pp.tile([128, 512], F32, tag="bank", name="swkps")
                nc.tensor.matmul(swk[0:32, :], perm_sb[0:32, 0:32],
                                 kq[0:32, :], start=True, stop=True)
                t1 = rp.tile([32, 512], F32, tag="t1k")
                nc.vector.tensor_mul(t1[:], kq[0:32, :], kab[0:32, :])
                t2 = rp.tile([32, 512], F32, tag="t2k")
                nc.vector.tensor_mul(t2[:], swk[0:32, :], kab[32:64, :])
                nc.vector.tensor_add(kq[0:32, :], t1[:], t2[:])
                nc.sync.dma_start(kv_loc[b][:, half * 512:(half + 1) * 512],
                                  kq[:])

            def kv_allgather(b):
                nc.gpsimd.collective_compute(
                    "AllGather", ALU.bypass,
                    replica_groups=[[0, 1, 2, 3], [4, 5, 6, 7]],
                    ins=[kv_loc[b][:].opt()],
                    outs=[kvg[b][:].opt()])

            def assemble_kv(b):
                """Rebuild K [128,1024] and V-transposed [tok,HD] from the
                gathered per-rank 64-col slices."""
                for r in range(4):
                    nc.scalar.dma_start(ktile[b][32 * r:32 * r + 32, :],
                                        kvg[b][r, 0:32, :])
                    nc.scalar.dma_start(vtile[b][32 * r:32 * r + 32, :],
                                        kvg[b][r, 32:64, :])
                for jj in range(2):
                    tp = pp.tile([128, 512], F16, tag="bank", name="vtrps")
                    for j in range(4):
                        nc.tensor.transpose(
                            tp[:, j * 128:(j + 1) * 128],
                            vtile[b][:, (jj * 4 + j) * 128:
                                     (jj * 4 + j + 1) * 128],
                            id_sb[:])
                    nc.scalar.copy(vn[b][:, jj * 512:(jj + 1) * 512], tp[:])

            def attn_batch(b, mid_hook=None):
                """Software-pipelined attention for one batch: flattened
                (qb, h, kt) stream with lookahead-2 sc -> rs/av."""
                units = [(qb, h) for qb in range(QB) for h in range(HPC)]
                tasks = []
                for u, (qb, h) in enumerate(units):
                    for kt in range(4 * (qb + 1)):
                        tasks.append((u, kt))
                n_kt = {u: 4 * (units[u][0] + 1) for u in range(len(units))}
                rs_ps, ctx_ps = {}, {}

                def emit_sc(u, kt):
                    qb, h = units[u]
                    tb = b * QB + qb
                    off = max(0, (kt - qb * 4) * 128)
                    N = 512 - off
                    sc = pp.tile([128, 512], F32, tag="bank", name="scps")
                    nc.tensor.matmul(sc[:, 0:N],
                                     ktile[b][:, kt * 128:(kt + 1) * 128],
                                     qtl[(h, tb)][:, off:512],
                                     start=True, stop=True)
                    e = ep.tile([128, 512], F16, tag="exp")
                    nc.scalar.activation(e[:, 0:N], sc[:, 0:N],
                                         AF.Exp, scale=SCALE)
                    if kt >= qb * 4:  # diagonal straddle: first 128 cols
                        nc.vector.tensor_mul(e[:, 0:128], e[:, 0:128],
                                             tri_sb[:])
                    return (e, off, N)

                def emit_rsav(u, kt, e, off, N):
                    qb, h = units[u]
                    if kt == 0:
                        rs_ps[u] = pp.tile([128, 512], F32, tag="bank",
                                           name="rsps")
                        ctx_ps[u] = pp.tile([128, 512], F32, tag="bank",
                                            name="ctxps")
                    first, last = kt == 0, kt == n_kt[u] - 1
                    nc.tensor.matmul(rs_ps[u][:, off:512], oc_sb[:],
                                     e[:, 0:N], start=first, stop=last)
                    nc.tensor.matmul(ctx_ps[u][:, off:512],
                                     vn[b][:, kt * 128:(kt + 1) * 128],
                                     e[:, 0:N], start=first, stop=last)
                    if last:
                        rcp = sp.tile([128, 512], F32, tag="rcp")
                        nc.vector.reciprocal_approx_fast(
                            out=rcp[:], in_=rs_ps[u][:])
                        ctxt = xcp.tile([128, 512], F16, tag="ctx")
                        nc.vector.tensor_mul(ctxt[:], ctx_ps[u][:], rcp[:])
                        for dup in range(4):
                            nc.gpsimd.dma_start(
                                a2a_in[b][qb * 4 + dup,
                                          h * 128:(h + 1) * 128, :],
                                ctxt[:])

                pend = {}
                mid_at = tasks.index((6, 0)) if mid_hook else -1
                for i, (u, kt) in enumerate(tasks):
                    if i == mid_at:
                        mid_hook()
                    pend[i] = (u, kt) + emit_sc(u, kt)
                    if i - 2 >= 0:
                        emit_rsav(*pend.pop(i - 2))
                for j in sorted(pend):
                    emit_rsav(*pend.pop(j))

            def a2a(b):
                nc.gpsimd.collective_compute(
                    "AllToAll", ALU.bypass,
                    replica_groups=[list(range(N_CORES))],
                    ins=[a2a_in[b][:].opt()],
                    outs=[a2a_out[b][:].opt()])

            for g in range(8):
                load_wg(g)
            proj_block(0)
            proj_block(1)
            kv_allgather(0)
            proj_block(2)
            assemble_kv(0)
            proj_block(3)
            kv_allgather(1)
            es2.close()

            for i in range(4):
                load_wd(i)
            attn_batch(0, mid_hook=lambda: assemble_kv(1))
            a2a(0)

            # gathered-context mega-tile: cg[:, kk, 0:512|512:1024]
            cgp = es.enter_context(
                tc.tile_pool(name="cg", bufs=1, side="right"))
            op_ = es.enter_context(
                tc.tile_pool(name="dout", bufs=4, side="right"))
            cg = cgp.tile([128, 32, 1024], F16, tag="cg", name="cg")
            cg0_src = a2a_out[0].rearrange("s (sub p) t -> p (s sub) t",
                                           sub=4)
            nc.scalar.dma_start(cg[:, :, 0:512], cg0_src[:])

            attn_batch(1)
            a2a(1)
            cg1_src = a2a_out[1].rearrange("s (sub p) t -> p (s sub) t",
                                           sub=4)
            nc.scalar.dma_start(cg[:, :, 512:1024], cg1_src[:])

            # dense: out[od 1024, tok 1024]; per odb two half-passes
            # (b0 cols / b1 cols), b1 passes lag one odb so the second
            # AllToAll is hidden behind ready b0 work.
            dps = {}
            osb = {}

            def dense_half(odb, half):
                if half == 0:
                    dps[odb] = [pp.tile([128, 512], F32, tag="bank",
                                        name=f"dops{odb}_{hh}")
                                for hh in range(2)]
                    osb[odb] = op_.tile([128, 1024], F32, tag="osb",
                                        name=f"osb{odb}")
                ps = dps[odb][half]
                for kk in range(32):
                    nc.tensor.matmul(
                        ps[:],
                        wd_sb[kk // 8][:, kk % 8,
                                       odb * 128:(odb + 1) * 128],
                        cg[:, kk, half * 512:(half + 1) * 512],
                        start=(kk == 0), stop=(kk == 31))
                nc.scalar.copy(osb[odb][:, half * 512:(half + 1) * 512],
                               ps[:])
                if half == 1:
                    nc.sync.dma_start(out_d[odb * 128:(odb + 1) * 128, :],
                                      osb[odb][:])

            dense_half(0, 0)
            dense_half(1, 0)
            for odb in range(2, 8):
                dense_half(odb - 2, 1)
                dense_half(odb, 0)
            dense_half(6, 1)
            dense_half(7, 1)
            es.close()

    nc.compile()
    return nc


_CACHE = {}


def _get_nc():
    if "nc" not in _CACHE:
        _CACHE["nc"] = build()
    return _CACHE["nc"]


def _host_prep(hidden_states, rope_cache, w_qkv, b_qkv, w_dense):
    """Build the 8 per-core input maps."""
    x = np.ascontiguousarray(hidden_states.reshape(TOK, H))
    xt = np.ascontiguousarray(x.T).astype(np.float16)

    # rope coefficient planes [64, TOK], token index j = b*S + s
    c0 = np.transpose(rope_cache[:, :, :, 0], (2, 1, 0)).reshape(ROT // 2, TOK)
    c1 = np.transpose(rope_cache[:, :, :, 1], (2, 1, 0)).reshape(ROT // 2, TOK)
    ra = np.repeat(c0, 2, axis=0).astype(np.float32)
    rb = np.repeat(c1, 2, axis=0).astype(np.float32)
    rb[0::2] *= -1.0
    rq = np.ascontiguousarray(np.vstack([ra, rb]))

    perm = np.zeros((ROT, ROT), np.float32)
    for k in range(ROT):
        perm[k, k ^ 1] = 1.0
    cc = np.zeros((128, 448), np.float32)
    cc[:, 0:128] = 1.0                                  # ones
    cc[:, 128:256] = np.triu(np.ones((128, 128)))       # tri[k,q]=1 iff q>=k
    cc[:, 256:384] = np.eye(128)                        # ident
    cc[0:64, 384:448] = perm
    cc = cc.astype(np.float16)

    in_maps = []
    for c in range(N_CORES):
        g = c // (N_CORES // G)     # KV group
        r = c % (N_CORES // G)      # rank within KV group
        oi = c % 4                  # dense output-quarter
        kc0 = NH * HD + g * HD + 32 * r          # K col slice start
        vc0 = NH * HD + G * HD + g * HD + 32 * r  # V col slice start
        wq_c = np.concatenate([
            w_qkv[:, c * DPC:(c + 1) * DPC],
            w_qkv[:, kc0:kc0 + 32],
            w_qkv[:, vc0:vc0 + 32],
        ], axis=1)
        bq_c = np.zeros((128, NDB), np.float32)
        bq_c[:, 0:4] = b_qkv[c * DPC:(c + 1) * DPC].reshape(4, 128).T
        bq_c[0:32, 4] = b_qkv[kc0:kc0 + 32]
        bq_c[32:64, 4] = b_qkv[vc0:vc0 + 32]
        if r < 2:
            rak = ra[32 * r:32 * r + 32]
            rbk = rb[32 * r:32 * r + 32]
        else:  # pass-dims: rope is identity
            rak = np.ones((32, TOK), np.float32)
            rbk = np.zeros((32, TOK), np.float32)
        in_maps.append({
            "xt": xt,
            "wqkv": wq_c.astype(np.float16),
            "bqkv": np.ascontiguousarray(bq_c),
            "consts": cc,
            "ropeQ": rq,
            "ropeK": np.ascontiguousarray(np.vstack([rak, rbk])),
            "wd": w_dense[:, oi * ODPC:(oi + 1) * ODPC].astype(np.float16),
        })
    return in_maps


def kernel(hidden_states, rope_cache, w_qkv, b_qkv, w_dense,
           _trace=False, _trace_cores=None):
    nc = _get_nc()
    in_maps = _host_prep(np.asarray(hidden_states), np.asarray(rope_cache),
                         np.asarray(w_qkv), np.asarray(b_qkv),
                         np.asarray(w_dense))
    res = run_bass_kernel_spmd(nc, in_maps, core_ids=list(range(N_CORES)),
                               trace=_trace, trace_cores=_trace_cores)
    _CACHE["last_result"] = res
    full = np.empty((TOK, H), np.float32)
    for c in range(N_CORES):
        ti, oi = c // 4, c % 4
        o = res.results[c]["out"]                 # [1024 od, 1024 tok]
        for b in range(B):
            full[b * S + ti * 512:b * S + (ti + 1) * 512,
                 oi * ODPC:(oi + 1) * ODPC] = o[:, b * 512:(b + 1) * 512].T
    return full.reshape(B, S, H)


# revision 22
# speedup vs baseline: 1.0630x; 1.0630x over previous
"""ChatGLM2 attention block on 8 Trainium2 NeuronCores (Bass/Tile).

Sharding: tensor-parallel across heads for QKV+attention; each core c owns
Q heads 4c..4c+3 (512 dims). K/V projection is split 4-ways inside each KV
group: core with group-rank r computes a 64-col [K 32 | V 32] slice, and a
small intra-group AllGather (128KB in) rebuilds the full K/V per batch.
Dense is sharded 2x4 (token-half x output-quarter): the per-core context is
redistributed with an AllToAll (stride-0-broadcast input, 2MB staged)
instead of a 16MB AllGather, and each core computes out[1024 od, 1024 tok]
against its w_dense column slice.

DMA triggers are the scarce resource (~0.7us dispatch each, serialized per
engine sequencer): they are batched into few large transfers (packed
consts, one DMA per weight group, packed rope planes, one mega-AP DMA for
the whole gathered-context tile) and spread across the gpsimd / vector /
scalar queues so the sync queue never backs up in front of a collective
trigger or a proj input load.

All matmuls run fp16 (1 cycle/row). Softmax skips the row-max (scores are
~1e-2); the denominator is a ones-matrix matmul accumulated alongside the
AV matmul; causal masking is a DVE multiply with a 128x128 triangular tile
on only the diagonal 128 columns of each straddling k-tile. The attention
inner loop is software-pipelined with lookahead 2 so the PE never waits on
the exp->mask chain.
"""

import math
import sys
import types

import numpy as np

# ---------------------------------------------------------------- constants
B, S, H = 2, 1024, 4096
NH, G, HD = 32, 2, 128
ROT = 64
N_CORES = 8
TOK = B * S                      # 2048
HPC = NH // N_CORES              # 4 Q heads per core
DPC = HPC * HD                   # 512 Q dims per core
NDB = 5                          # per-core qkv dim blocks: 4x Q(128) + KV(64)
WQC = DPC + 64                   # 576 qkv cols per core
TB = 4                           # token blocks of 512
QB = 2                           # q blocks of 512 per batch
ODPC = H // 4                    # 1024 output dims per core (dense quarter)
SCALE = 1.0 / math.sqrt(HD)


def _install_ntff_hook():
    """The agent image's antenv lacks axon_hooks; shim it so
    run_bass_kernel_spmd(trace=True) can profile via NTFF."""
    if "antenv.axon_hooks" in sys.modules:
        return
    import antenv  # noqa: F401

    mod = types.ModuleType("antenv.axon_hooks")
    mod._hook = None
    mod.set_axon_ntff_profile_hook = lambda h: setattr(mod, "_hook", h)
    mod.get_axon_ntff_profile_hook = lambda: mod._hook
    sys.modules["antenv.axon_hooks"] = mod
    try:
        from trn_agent_boot.trn_boot import _ntff_profile_via_ctypes

        mod._hook = _ntff_profile_via_ctypes("/opt/axon/libaxon_pjrt.so")
    except Exception:
        pass


_install_ntff_hook()

import concourse.bass as bass  # noqa: E402
import concourse.mybir as mybir  # noqa: E402
import concourse.tile as tile  # noqa: E402
from concourse import bacc  # noqa: E402
from concourse.bass_utils import run_bass_kernel_spmd  # noqa: E402

F32 = mybir.dt.float32
F16 = mybir.dt.float16
AF = mybir.ActivationFunctionType
ALU = mybir.AluOpType


# ---------------------------------------------------------------- build
def build(trace_label="k"):
    nc = bacc.Bacc("TRN2", target_bir_lowering=False, debug=False,
                   num_devices=N_CORES)

    xt_d = nc.dram_tensor("xt", [H, TOK], F16, kind="ExternalInput").ap()
    wq_d = nc.dram_tensor("wqkv", [H, WQC], F16, kind="ExternalInput").ap()
    bq_d = nc.dram_tensor("bqkv", [128, NDB], F32, kind="ExternalInput").ap()
    # packed f16 consts: [ ones(128) | tri(128) | ident(128) | perm(64c) ]
    cc_d = nc.dram_tensor("consts", [128, 448], F16, kind="ExternalInput").ap()
    # packed rope planes: ropeQ = [ra;rb] (128 rows), ropeK = [rak;rbk] (64)
    rq_d = nc.dram_tensor("ropeQ", [128, TOK], F32, kind="ExternalInput").ap()
    rk_d = nc.dram_tensor("ropeK", [64, TOK], F32, kind="ExternalInput").ap()
    wd_d = nc.dram_tensor("wd", [H, ODPC], F16, kind="ExternalInput").ap()
    out_d = nc.dram_tensor("out", [ODPC, 1024], F32, kind="ExternalOutput").ap()

    from contextlib import ExitStack

    with tile.TileContext(nc) as tc:
        with tc.tile_pool(name="consts", bufs=1) as cp, \
             tc.tile_pool(name="dram", bufs=1, space="DRAM") as dp:
            # ---- small constants (alive whole kernel)
            bias_sb = cp.tile([128, NDB], F32, tag="bias")
            cc_sb = cp.tile([128, 448], F16, tag="consts")
            nc.sync.dma_start(bias_sb[:], bq_d[:])
            nc.sync.dma_start(cc_sb[:], cc_d[:])
            oc_sb = cc_sb[:, 0:128]
            tri_sb = cc_sb[:, 128:256]
            id_sb = cc_sb[:, 256:384]
            perm_sb = cc_sb[0:64, 384:448]

            # DRAM staging for collectives
            kv_loc = [dp.tile([64, 1024], F16, tag=f"kvl{b}",
                              name=f"kvl{b}") for b in range(B)]
            kvg = [dp.tile([4, 64, 1024], F16, tag=f"kvg{b}",
                           name=f"kvg{b}") for b in range(B)]
            a2a_in = [dp.tile([8, 512, 512], F16, tag=f"a2i{b}",
                              name=f"a2i{b}") for b in range(B)]
            a2a_out = [dp.tile([8, 512, 512], F16, tag=f"a2o{b}",
                               name=f"a2o{b}") for b in range(B)]

            wq_r = wq_d.rearrange("(k p) d -> p k d", p=128)
            xt_r = xt_d.rearrange("(k p) t -> p k t", p=128)
            wd_r = wd_d.rearrange("(k p) n -> p k n", p=128)

            es = ExitStack()
            pp = es.enter_context(
                tc.tile_pool(name="ps_main", bufs=8, space="PSUM"))
            kp = es.enter_context(tc.tile_pool(name="kvp", bufs=1))
            qtp1 = es.enter_context(tc.tile_pool(name="qtp1", bufs=1))
            xcp = es.enter_context(tc.tile_pool(name="ctxp", bufs=5))
            ep = es.enter_context(tc.tile_pool(name="exp", bufs=8))
            sp = es.enter_context(tc.tile_pool(name="att_small", bufs=2))
            wdp = es.enter_context(
                tc.tile_pool(name="wd", bufs=1, side="right"))
            qt0 = ExitStack()
            qtp0 = qt0.enter_context(tc.tile_pool(name="qtp0", bufs=1))
            es2 = ExitStack()
            wp = es2.enter_context(tc.tile_pool(name="wq", bufs=1))
            xp = es2.enter_context(tc.tile_pool(name="xt", bufs=3))
            rp = es2.enter_context(tc.tile_pool(name="rope_tmp", bufs=2))
            abp = es2.enter_context(tc.tile_pool(name="ropeab", bufs=2))
            kqp = es2.enter_context(tc.tile_pool(name="kq", bufs=2))

            # per-batch K / V(transposed) tiles
            ktile = [kp.tile([128, 1024], F16, tag=f"k{b}", name=f"k{b}")
                     for b in range(B)]
            vtile = [kp.tile([128, 1024], F16, tag=f"vt{b}", name=f"vt{b}")
                     for b in range(B)]
            vn = [kp.tile([128, 1024], F16, tag=f"vn{b}", name=f"vn{b}")
                  for b in range(B)]
            qtl = {}
            w_sb = [None] * 8
            wd_sb = []

            def load_wg(g):
                if w_sb[g] is None:
                    wg = wp.tile([128, 4, WQC], F16,
                                 tag=f"wq{g}", name=f"wq{g}")
                    nc.sync.dma_start(wg[:], wq_r[:, g * 4:(g + 1) * 4, :])
                    w_sb[g] = wg

            def load_wd(i):
                wg = wdp.tile([128, 8, ODPC], F16, tag=f"wd{i}",
                              name=f"wdg{i}")
                nc.sync.dma_start(wg[:], wd_r[:, i * 8:(i + 1) * 8, :])
                wd_sb.append(wg)

            def proj_block(t):
                """QKV^T projection + bias + RoPE for one 512-token block.
                Q: 4x128 dims; KV: 64 dims ([K32|V32], rank slice)."""
                b, half = t // 2, t % 2
                ps = [pp.tile([128, 512], F32, tag="bank",
                              name=f"qkvps{d}") for d in range(NDB)]
                for gg in range(4):
                    xg = xp.tile([128, 8, 512], F16, tag="xtblk")
                    nc.sync.dma_start(
                        xg[:], xt_r[:, gg * 8:(gg + 1) * 8,
                                    t * 512:(t + 1) * 512])
                    for kk in range(8):
                        g, k = (gg * 8 + kk) // 4, kk % 4
                        for d in range(4):
                            nc.tensor.matmul(
                                ps[d][:],
                                w_sb[g][:, k, d * 128:(d + 1) * 128],
                                xg[:, kk, :],
                                start=(gg == 0 and kk == 0),
                                stop=(gg == 3 and kk == 7),
                            )
                        nc.tensor.matmul(
                            ps[4][0:64, :],
                            w_sb[g][:, k, 512:576],
                            xg[:, kk, :],
                            start=(gg == 0 and kk == 0),
                            stop=(gg == 3 and kk == 7),
                        )
                for h in range(HPC):
                    qt = (qtp0 if t < 2 else qtp1).tile(
                        [128, 512], F16, tag=f"q{h}_{t}", name=f"q{h}_{t}")
                    qtl[(h, t)] = qt
                    nc.scalar.activation(qt[:], ps[h][:], AF.Identity,
                                         bias=bias_sb[:, h:h + 1])
                kq = kqp.tile([64, 512], F16, tag="kvtile", name=f"kv{t}")
                nc.scalar.activation(kq[:], ps[4][0:64, :], AF.Identity,
                                     bias=bias_sb[0:64, 4:5])
                tsl = slice(t * 512, (t + 1) * 512)
                qab = abp.tile([128, 512], F32, tag="ropeQb")
                nc.sync.dma_start(qab[:], rq_d[:, tsl])
                kab = abp.tile([64, 512], F32, tag="ropeKb", bufs=1)
                nc.sync.dma_start(kab[:], rk_d[:, tsl])
                for e in range(HPC):
                    qt = qtl[(e, t)]
                    sw = pp.tile([128, 512], F32, tag="bank", name="swps")
                    nc.tensor.matmul(sw[0:ROT, :], perm_sb[:],
                                     qt[0:ROT, :], start=True, stop=True)
                    t1 = rp.tile([ROT, 512], F32, tag="t1")
                    nc.vector.tensor_mul(t1[:], qt[0:ROT, :], qab[0:64, :])
                    t2 = rp.tile([ROT, 512], F32, tag="t2")
                    nc.vector.tensor_mul(t2[:], sw[0:ROT, :], qab[64:128, :])
                    nc.vector.tensor_add(qt[0:ROT, :], t1[:], t2[:])
                # K slice rope (identity planes on ranks holding pass-dims)
                swk = pp.tile([128, 512], F32, tag="bank", name="swkps")
                nc.tensor.matmul(swk[0:32, :], perm_sb[0:32, 0:32],
                                 kq[0:32, :], start=True, stop=True)
                t1 = rp.tile([32, 512], F32, tag="t1k", bufs=1)
                nc.vector.tensor_mul(t1[:], kq[0:32, :], kab[0:32, :])
                t2 = rp.tile([32, 512], F32, tag="t2k", bufs=1)
                nc.vector.tensor_mul(t2[:], swk[0:32, :], kab[32:64, :])
                nc.vector.tensor_add(kq[0:32, :], t1[:], t2[:])
                nc.sync.dma_start(kv_loc[b][:, half * 512:(half + 1) * 512],
                                  kq[:])

            def kv_allgather(b):
                nc.gpsimd.collective_compute(
                    "AllGather", ALU.bypass,
                    replica_groups=[[0, 1, 2, 3], [4, 5, 6, 7]],
                    ins=[kv_loc[b][:].opt()],
                    outs=[kvg[b][:].opt()])

            def assemble_kv(b):
                """Rebuild K [128,1024] and V-transposed [tok,HD] from the
                gathered per-rank 64-col slices."""
                for r in range(4):
                    nc.gpsimd.dma_start(ktile[b][32 * r:32 * r + 32, :],
                                        kvg[b][r, 0:32, :])
                    nc.gpsimd.dma_start(vtile[b][32 * r:32 * r + 32, :],
                                        kvg[b][r, 32:64, :])
                for jj in range(2):
                    tp = pp.tile([128, 512], F16, tag="bank", name="vtrps")
                    for j in range(4):
                        nc.tensor.transpose(
                            tp[:, j * 128:(j + 1) * 128],
                            vtile[b][:, (jj * 4 + j) * 128:
                                     (jj * 4 + j + 1) * 128],
                            id_sb[:])
                    nc.scalar.copy(vn[b][:, jj * 512:(jj + 1) * 512], tp[:])

            def attn_batch(b, mid_hook=None):
                """Software-pipelined attention for one batch: flattened
                (qb, h, kt) stream with lookahead-2 sc -> rs/av."""
                units = [(qb, h) for qb in range(QB) for h in range(HPC)]
                tasks = []
                for u, (qb, h) in enumerate(units):
                    for kt in range(4 * (qb + 1)):
                        tasks.append((u, kt))
                n_kt = {u: 4 * (units[u][0] + 1) for u in range(len(units))}
                rs_ps, ctx_ps = {}, {}

                def emit_sc(u, kt):
                    qb, h = units[u]
                    tb = b * QB + qb
                    off = max(0, (kt - qb * 4) * 128)
                    N = 512 - off
                    sc = pp.tile([128, 512], F32, tag="bank", name="scps")
                    nc.tensor.matmul(sc[:, 0:N],
                                     ktile[b][:, kt * 128:(kt + 1) * 128],
                                     qtl[(h, tb)][:, off:512],
                                     start=True, stop=True)
                    e = ep.tile([128, 512], F16, tag="exp")
                    nc.scalar.activation(e[:, 0:N], sc[:, 0:N],
                                         AF.Exp, scale=SCALE)
                    if kt >= qb * 4:  # diagonal straddle: first 128 cols
                        nc.vector.tensor_mul(e[:, 0:128], e[:, 0:128],
                                             tri_sb[:])
                    return (e, off, N)

                def emit_rsav(u, kt, e, off, N):
                    qb, h = units[u]
                    if kt == 0:
                        rs_ps[u] = pp.tile([128, 512], F32, tag="bank",
                                           name="rsps")
                        ctx_ps[u] = pp.tile([128, 512], F32, tag="bank",
                                            name="ctxps")
                    first, last = kt == 0, kt == n_kt[u] - 1
                    nc.tensor.matmul(rs_ps[u][:, off:512], oc_sb[:],
                                     e[:, 0:N], start=first, stop=last)
                    nc.tensor.matmul(ctx_ps[u][:, off:512],
                                     vn[b][:, kt * 128:(kt + 1) * 128],
                                     e[:, 0:N], start=first, stop=last)
                    if last:
                        rcp = sp.tile([128, 512], F32, tag="rcp")
                        nc.vector.reciprocal_approx_fast(
                            out=rcp[:], in_=rs_ps[u][:])
                        ctxt = xcp.tile([128, 512], F16, tag="ctx")
                        nc.vector.tensor_mul(ctxt[:], ctx_ps[u][:], rcp[:])
                        # one DMA writes all 4 dup blocks: src broadcasts
                        # via a stride-0 dim after the partition dim
                        cap = ctxt[:]
                        bsrc = bass.AP(cap.tensor, cap.offset,
                                       [cap.ap[0], (0, 4), cap.ap[1]])
                        dst = a2a_in[b].rearrange("j p t -> p j t")[
                            h * 128:(h + 1) * 128, qb * 4:qb * 4 + 4, :]
                        nc.sync.dma_start(dst, bsrc)

                pend = {}
                mid_at = tasks.index((6, 0)) if mid_hook else -1
                for i, (u, kt) in enumerate(tasks):
                    if i == mid_at:
                        mid_hook()
                    pend[i] = (u, kt) + emit_sc(u, kt)
                    if i - 2 >= 0:
                        emit_rsav(*pend.pop(i - 2))
                for j in sorted(pend):
                    emit_rsav(*pend.pop(j))

            def a2a(b):
                nc.gpsimd.collective_compute(
                    "AllToAll", ALU.bypass,
                    replica_groups=[list(range(N_CORES))],
                    ins=[a2a_in[b][:].opt()],
                    outs=[a2a_out[b][:].opt()])

            for g in range(8):
                load_wg(g)
            proj_block(0)
            proj_block(1)
            kv_allgather(0)
            proj_block(2)
            assemble_kv(0)
            proj_block(3)
            kv_allgather(1)
            es2.close()

            for i in range(4):
                load_wd(i)
            attn_batch(0, mid_hook=lambda: assemble_kv(1))
            a2a(0)
            qt0.close()

            # gathered-context mega-tile: cg[:, kk, 0:512|512:1024],
            # loaded in 8-kk chunks so dense can start on the first chunk
            cgp = es.enter_context(
                tc.tile_pool(name="cg", bufs=1, side="right"))
            op_ = es.enter_context(
                tc.tile_pool(name="dout", bufs=4, side="right"))
            cg = cgp.tile([128, 32, 1024], F16, tag="cg", name="cg")
            cg0_src = a2a_out[0].rearrange("s (sub p) t -> p (s sub) t",
                                           sub=4)
            for cc in range(4):
                nc.sync.dma_start(cg[:, cc * 8:(cc + 1) * 8, 0:512],
                                  cg0_src[:, cc * 8:(cc + 1) * 8, :])

            attn_batch(1)
            a2a(1)
            cg1_src = a2a_out[1].rearrange("s (sub p) t -> p (s sub) t",
                                           sub=4)
            for cc in range(4):
                nc.sync.dma_start(cg[:, cc * 8:(cc + 1) * 8, 512:1024],
                                  cg1_src[:, cc * 8:(cc + 1) * 8, :])

            # dense: out[od 1024, tok 1024]; all batch-0 half-passes first
            # (55us of work independent of the second AllToAll), then the
            # batch-1 half-passes.
            def dense_half(odb, half):
                ps = pp.tile([128, 512], F32, tag="bank",
                             name=f"dops{odb}_{half}")
                for kk in range(32):
                    nc.tensor.matmul(
                        ps[:],
                        wd_sb[kk // 8][:, kk % 8,
                                       odb * 128:(odb + 1) * 128],
                        cg[:, kk, half * 512:(half + 1) * 512],
                        start=(kk == 0), stop=(kk == 31))
                o = op_.tile([128, 512], F32, tag="osb")
                nc.scalar.copy(o[:], ps[:])
                nc.sync.dma_start(
                    out_d[odb * 128:(odb + 1) * 128,
                          half * 512:(half + 1) * 512], o[:])

            for odb in range(8):
                dense_half(odb, 0)
            for odb in range(8):
                dense_half(odb, 1)
            es.close()

    nc.compile()
    return nc


_CACHE = {}


def _get_nc():
    if "nc" not in _CACHE:
        _CACHE["nc"] = build()
    return _CACHE["nc"]


def _host_prep(hidden_states, rope_cache, w_qkv, b_qkv, w_dense):
    """Build the 8 per-core input maps."""
    x = np.ascontiguousarray(hidden_states.reshape(TOK, H))
    xt = np.ascontiguousarray(x.T).astype(np.float16)

    # rope coefficient planes [64, TOK], token index j = b*S + s
    c0 = np.transpose(rope_cache[:, :, :, 0], (2, 1, 0)).reshape(ROT // 2, TOK)
    c1 = np.transpose(rope_cache[:, :, :, 1], (2, 1, 0)).reshape(ROT // 2, TOK)
    ra = np.repeat(c0, 2, axis=0).astype(np.float32)
    rb = np.repeat(c1, 2, axis=0).astype(np.float32)
    rb[0::2] *= -1.0
    rq = np.ascontiguousarray(np.vstack([ra, rb]))

    perm = np.zeros((ROT, ROT), np.float32)
    for k in range(ROT):
        perm[k, k ^ 1] = 1.0
    cc = np.zeros((128, 448), np.float32)
    cc[:, 0:128] = 1.0                                  # ones
    cc[:, 128:256] = np.triu(np.ones((128, 128)))       # tri[k,q]=1 iff q>=k
    cc[:, 256:384] = np.eye(128)                        # ident
    cc[0:64, 384:448] = perm
    cc = cc.astype(np.float16)

    in_maps = []
    for c in range(N_CORES):
        g = c // (N_CORES // G)     # KV group
        r = c % (N_CORES // G)      # rank within KV group
        oi = c % 4                  # dense output-quarter
        kc0 = NH * HD + g * HD + 32 * r          # K col slice start
        vc0 = NH * HD + G * HD + g * HD + 32 * r  # V col slice start
        wq_c = np.concatenate([
            w_qkv[:, c * DPC:(c + 1) * DPC],
            w_qkv[:, kc0:kc0 + 32],
            w_qkv[:, vc0:vc0 + 32],
        ], axis=1)
        bq_c = np.zeros((128, NDB), np.float32)
        bq_c[:, 0:4] = b_qkv[c * DPC:(c + 1) * DPC].reshape(4, 128).T
        bq_c[0:32, 4] = b_qkv[kc0:kc0 + 32]
        bq_c[32:64, 4] = b_qkv[vc0:vc0 + 32]
        if r < 2:
            rak = ra[32 * r:32 * r + 32]
            rbk = rb[32 * r:32 * r + 32]
        else:  # pass-dims: rope is identity
            rak = np.ones((32, TOK), np.float32)
            rbk = np.zeros((32, TOK), np.float32)
        in_maps.append({
            "xt": xt,
            "wqkv": wq_c.astype(np.float16),
            "bqkv": np.ascontiguousarray(bq_c),
            "consts": cc,
            "ropeQ": rq,
            "ropeK": np.ascontiguousarray(np.vstack([rak, rbk])),
            "wd": w_dense[:, oi * ODPC:(oi + 1) * ODPC].astype(np.float16),
        })
    return in_maps


def kernel(hidden_states, rope_cache, w_qkv, b_qkv, w_dense,
           _trace=False, _trace_cores=None):
    nc = _get_nc()
    in_maps = _host_prep(np.asarray(hidden_states), np.asarray(rope_cache),
                         np.asarray(w_qkv), np.asarray(b_qkv),
                         np.asarray(w_dense))
    res = run_bass_kernel_spmd(nc, in_maps, core_ids=list(range(N_CORES)),
                               trace=_trace, trace_cores=_trace_cores)
    _CACHE["last_result"] = res
    full = np.empty((TOK, H), np.float32)
    for c in range(N_CORES):
        ti, oi = c // 4, c % 4
        o = res.results[c]["out"]                 # [1024 od, 1024 tok]
        for b in range(B):
            full[b * S + ti * 512:b * S + (ti + 1) * 512,
                 oi * ODPC:(oi + 1) * ODPC] = o[:, b * 512:(b + 1) * 512].T
    return full.reshape(B, S, H)


# revision 23
# speedup vs baseline: 1.0746x; 1.0109x over previous
"""ChatGLM2 attention block on 8 Trainium2 NeuronCores (Bass/Tile).

Sharding: tensor-parallel across heads for QKV+attention; each core c owns
Q heads 4c..4c+3 (512 dims). K/V projection is split 4-ways inside each KV
group: core with group-rank r computes a 64-col [K 32 | V 32] slice, and a
small intra-group AllGather (128KB in) rebuilds the full K/V per batch.
Dense is sharded 2x4 (token-half x output-quarter): the per-core context is
redistributed with an AllToAll (stride-0-broadcast input, 2MB staged)
instead of a 16MB AllGather, and each core computes out[1024 od, 1024 tok]
against its w_dense column slice.

DMA triggers are the scarce resource (~0.7us dispatch each, serialized per
engine sequencer): they are batched into few large transfers (packed
consts, one DMA per weight group, packed rope planes, one mega-AP DMA for
the whole gathered-context tile) and spread across the gpsimd / vector /
scalar queues so the sync queue never backs up in front of a collective
trigger or a proj input load.

All matmuls run fp16 (1 cycle/row). Softmax skips the row-max (scores are
~1e-2); the denominator is a ones-matrix matmul accumulated alongside the
AV matmul; causal masking is a DVE multiply with a 128x128 triangular tile
on only the diagonal 128 columns of each straddling k-tile. The attention
inner loop is software-pipelined with lookahead 2 so the PE never waits on
the exp->mask chain.
"""

import math
import sys
import types

import numpy as np

# ---------------------------------------------------------------- constants
B, S, H = 2, 1024, 4096
NH, G, HD = 32, 2, 128
ROT = 64
N_CORES = 8
TOK = B * S                      # 2048
HPC = NH // N_CORES              # 4 Q heads per core
DPC = HPC * HD                   # 512 Q dims per core
NDB = 5                          # per-core qkv dim blocks: 4x Q(128) + KV(64)
WQC = DPC + 64                   # 576 qkv cols per core
TB = 4                           # token blocks of 512
QB = 2                           # q blocks of 512 per batch
ODPC = H // 4                    # 1024 output dims per core (dense quarter)
SCALE = 1.0 / math.sqrt(HD)


def _install_ntff_hook():
    """The agent image's antenv lacks axon_hooks; shim it so
    run_bass_kernel_spmd(trace=True) can profile via NTFF."""
    if "antenv.axon_hooks" in sys.modules:
        return
    import antenv  # noqa: F401

    mod = types.ModuleType("antenv.axon_hooks")
    mod._hook = None
    mod.set_axon_ntff_profile_hook = lambda h: setattr(mod, "_hook", h)
    mod.get_axon_ntff_profile_hook = lambda: mod._hook
    sys.modules["antenv.axon_hooks"] = mod
    try:
        from trn_agent_boot.trn_boot import _ntff_profile_via_ctypes

        mod._hook = _ntff_profile_via_ctypes("/opt/axon/libaxon_pjrt.so")
    except Exception:
        pass


_install_ntff_hook()

import concourse.bass as bass  # noqa: E402
import concourse.mybir as mybir  # noqa: E402
import concourse.tile as tile  # noqa: E402
from concourse import bacc  # noqa: E402
from concourse.bass_utils import run_bass_kernel_spmd  # noqa: E402

F32 = mybir.dt.float32
F16 = mybir.dt.float16
AF = mybir.ActivationFunctionType
ALU = mybir.AluOpType


# ---------------------------------------------------------------- build
def build(trace_label="k"):
    nc = bacc.Bacc("TRN2", target_bir_lowering=False, debug=False,
                   num_devices=N_CORES)

    xt_d = nc.dram_tensor("xt", [H, TOK], F16, kind="ExternalInput").ap()
    wq_d = nc.dram_tensor("wqkv", [H, WQC], F16, kind="ExternalInput").ap()
    bq_d = nc.dram_tensor("bqkv", [128, NDB], F32, kind="ExternalInput").ap()
    # packed f16 consts: [ ones(128) | tri(128) | ident(128) | perm(64c) ]
    cc_d = nc.dram_tensor("consts", [128, 448], F16, kind="ExternalInput").ap()
    # packed rope planes: ropeQ = [ra;rb] (128 rows), ropeK = [rak;rbk] (64)
    rq_d = nc.dram_tensor("ropeQ", [128, TOK], F32, kind="ExternalInput").ap()
    rk_d = nc.dram_tensor("ropeK", [64, TOK], F32, kind="ExternalInput").ap()
    wd_d = nc.dram_tensor("wd", [H, ODPC], F16, kind="ExternalInput").ap()
    out_d = nc.dram_tensor("out", [ODPC, 1024], F32, kind="ExternalOutput").ap()

    from contextlib import ExitStack

    with tile.TileContext(nc) as tc:
        with tc.tile_pool(name="consts", bufs=1) as cp, \
             tc.tile_pool(name="dram", bufs=1, space="DRAM") as dp:
            # ---- small constants (alive whole kernel)
            bias_sb = cp.tile([128, NDB], F32, tag="bias")
            cc_sb = cp.tile([128, 448], F16, tag="consts")
            nc.sync.dma_start(bias_sb[:], bq_d[:])
            nc.sync.dma_start(cc_sb[:], cc_d[:])
            oc_sb = cc_sb[:, 0:128]
            tri_sb = cc_sb[:, 128:256]
            id_sb = cc_sb[:, 256:384]
            perm_sb = cc_sb[0:64, 384:448]

            # DRAM staging for collectives
            kv_loc = [dp.tile([64, 1024], F16, tag=f"kvl{b}",
                              name=f"kvl{b}") for b in range(B)]
            kvg = [dp.tile([4, 64, 1024], F16, tag=f"kvg{b}",
                           name=f"kvg{b}") for b in range(B)]
            a2a_in = [dp.tile([8, 512, 512], F16, tag=f"a2i{b}",
                              name=f"a2i{b}") for b in range(B)]
            a2a_out = [dp.tile([8, 512, 512], F16, tag=f"a2o{b}",
                               name=f"a2o{b}") for b in range(B)]

            wq_r = wq_d.rearrange("(k p) d -> p k d", p=128)
            xt_r = xt_d.rearrange("(k p) t -> p k t", p=128)
            wd_r = wd_d.rearrange("(k p) n -> p k n", p=128)

            es = ExitStack()
            pp = es.enter_context(
                tc.tile_pool(name="ps_main", bufs=8, space="PSUM"))
            kp = es.enter_context(tc.tile_pool(name="kvp", bufs=1))
            qtp1 = es.enter_context(tc.tile_pool(name="qtp1", bufs=1))
            xcp = es.enter_context(tc.tile_pool(name="ctxp", bufs=5))
            ep = es.enter_context(tc.tile_pool(name="exp", bufs=8))
            sp = es.enter_context(tc.tile_pool(name="att_small", bufs=2))
            wdp = es.enter_context(
                tc.tile_pool(name="wd", bufs=1, side="right"))
            qt0 = ExitStack()
            qtp0 = qt0.enter_context(tc.tile_pool(name="qtp0", bufs=1))
            es2 = ExitStack()
            wp = es2.enter_context(tc.tile_pool(name="wq", bufs=1))
            xp = es2.enter_context(tc.tile_pool(name="xt", bufs=4))
            rp = es2.enter_context(tc.tile_pool(name="rope_tmp", bufs=2))
            abp = es2.enter_context(tc.tile_pool(name="ropeab", bufs=2))
            kqp = es2.enter_context(tc.tile_pool(name="kq", bufs=2))

            # per-batch K / V(transposed) tiles
            ktile = [kp.tile([128, 1024], F16, tag=f"k{b}", name=f"k{b}")
                     for b in range(B)]
            vtile = [kp.tile([128, 1024], F16, tag=f"vt{b}", name=f"vt{b}")
                     for b in range(B)]
            vn = [kp.tile([128, 1024], F16, tag=f"vn{b}", name=f"vn{b}")
                  for b in range(B)]
            qtl = {}
            w_sb = [None] * 8
            wd_sb = []

            def load_wg(g):
                if w_sb[g] is None:
                    wg = wp.tile([128, 4, WQC], F16,
                                 tag=f"wq{g}", name=f"wq{g}")
                    nc.sync.dma_start(wg[:], wq_r[:, g * 4:(g + 1) * 4, :])
                    w_sb[g] = wg

            def load_wd(i):
                wg = wdp.tile([128, 8, ODPC], F16, tag=f"wd{i}",
                              name=f"wdg{i}")
                nc.sync.dma_start(wg[:], wd_r[:, i * 8:(i + 1) * 8, :])
                wd_sb.append(wg)

            def proj_block(t):
                """QKV^T projection + bias + RoPE for one 512-token block.
                Q: 4x128 dims; KV: 64 dims ([K32|V32], rank slice)."""
                b, half = t // 2, t % 2
                ps = [pp.tile([128, 512], F32, tag="bank",
                              name=f"qkvps{d}") for d in range(NDB)]
                for gg in range(4):
                    xg = xp.tile([128, 8, 512], F16, tag="xtblk")
                    nc.sync.dma_start(
                        xg[:], xt_r[:, gg * 8:(gg + 1) * 8,
                                    t * 512:(t + 1) * 512])
                    if t == 0 and gg < 3:
                        load_wg(gg * 2 + 2)
                        load_wg(gg * 2 + 3)
                    for kk in range(8):
                        g, k = (gg * 8 + kk) // 4, kk % 4
                        for d in range(4):
                            nc.tensor.matmul(
                                ps[d][:],
                                w_sb[g][:, k, d * 128:(d + 1) * 128],
                                xg[:, kk, :],
                                start=(gg == 0 and kk == 0),
                                stop=(gg == 3 and kk == 7),
                            )
                        nc.tensor.matmul(
                            ps[4][0:64, :],
                            w_sb[g][:, k, 512:576],
                            xg[:, kk, :],
                            start=(gg == 0 and kk == 0),
                            stop=(gg == 3 and kk == 7),
                        )
                for h in range(HPC):
                    qt = (qtp0 if t < 2 else qtp1).tile(
                        [128, 512], F16, tag=f"q{h}_{t}", name=f"q{h}_{t}")
                    qtl[(h, t)] = qt
                    nc.scalar.activation(qt[:], ps[h][:], AF.Identity,
                                         bias=bias_sb[:, h:h + 1])
                kq = kqp.tile([64, 512], F16, tag="kvtile", name=f"kv{t}")
                nc.scalar.activation(kq[:], ps[4][0:64, :], AF.Identity,
                                     bias=bias_sb[0:64, 4:5])
                tsl = slice(t * 512, (t + 1) * 512)
                qab = abp.tile([128, 512], F32, tag="ropeQb")
                nc.sync.dma_start(qab[:], rq_d[:, tsl])
                kab = abp.tile([64, 512], F32, tag="ropeKb", bufs=1)
                nc.sync.dma_start(kab[:], rk_d[:, tsl])
                for e in range(HPC):
                    qt = qtl[(e, t)]
                    sw = pp.tile([128, 512], F32, tag="bank", name="swps")
                    nc.tensor.matmul(sw[0:ROT, :], perm_sb[:],
                                     qt[0:ROT, :], start=True, stop=True)
                    t1 = rp.tile([ROT, 512], F32, tag="t1")
                    nc.vector.tensor_mul(t1[:], qt[0:ROT, :], qab[0:64, :])
                    t2 = rp.tile([ROT, 512], F32, tag="t2")
                    nc.vector.tensor_mul(t2[:], sw[0:ROT, :], qab[64:128, :])
                    nc.vector.tensor_add(qt[0:ROT, :], t1[:], t2[:])
                # K slice rope (identity planes on ranks holding pass-dims)
                swk = pp.tile([128, 512], F32, tag="bank", name="swkps")
                nc.tensor.matmul(swk[0:32, :], perm_sb[0:32, 0:32],
                                 kq[0:32, :], start=True, stop=True)
                t1 = rp.tile([32, 512], F32, tag="t1k", bufs=1)
                nc.vector.tensor_mul(t1[:], kq[0:32, :], kab[0:32, :])
                t2 = rp.tile([32, 512], F32, tag="t2k", bufs=1)
                nc.vector.tensor_mul(t2[:], swk[0:32, :], kab[32:64, :])
                nc.vector.tensor_add(kq[0:32, :], t1[:], t2[:])
                nc.scalar.dma_start(
                    kv_loc[b][:, half * 512:(half + 1) * 512], kq[:])

            def kv_allgather(b):
                nc.gpsimd.collective_compute(
                    "AllGather", ALU.bypass,
                    replica_groups=[[0, 1, 2, 3], [4, 5, 6, 7]],
                    ins=[kv_loc[b][:].opt()],
                    outs=[kvg[b][:].opt()])

            def assemble_kv(b):
                """Rebuild K [128,1024] and V-transposed [tok,HD] from the
                gathered per-rank 64-col slices."""
                for r in range(4):
                    nc.gpsimd.dma_start(ktile[b][32 * r:32 * r + 32, :],
                                        kvg[b][r, 0:32, :])
                    nc.gpsimd.dma_start(vtile[b][32 * r:32 * r + 32, :],
                                        kvg[b][r, 32:64, :])
                for jj in range(2):
                    tp = pp.tile([128, 512], F16, tag="bank", name="vtrps")
                    for j in range(4):
                        nc.tensor.transpose(
                            tp[:, j * 128:(j + 1) * 128],
                            vtile[b][:, (jj * 4 + j) * 128:
                                     (jj * 4 + j + 1) * 128],
                            id_sb[:])
                    nc.scalar.copy(vn[b][:, jj * 512:(jj + 1) * 512], tp[:])

            def attn_batch(b, mid_hook=None):
                """Software-pipelined attention for one batch: flattened
                (qb, h, kt) stream with lookahead-2 sc -> rs/av."""
                units = [(qb, h) for qb in range(QB) for h in range(HPC)]
                tasks = []
                for u, (qb, h) in enumerate(units):
                    for kt in range(4 * (qb + 1)):
                        tasks.append((u, kt))
                n_kt = {u: 4 * (units[u][0] + 1) for u in range(len(units))}
                rs_ps, ctx_ps = {}, {}

                def emit_sc(u, kt):
                    qb, h = units[u]
                    tb = b * QB + qb
                    off = max(0, (kt - qb * 4) * 128)
                    N = 512 - off
                    sc = pp.tile([128, 512], F32, tag="bank", name="scps")
                    nc.tensor.matmul(sc[:, 0:N],
                                     ktile[b][:, kt * 128:(kt + 1) * 128],
                                     qtl[(h, tb)][:, off:512],
                                     start=True, stop=True)
                    e = ep.tile([128, 512], F16, tag="exp")
                    nc.scalar.activation(e[:, 0:N], sc[:, 0:N],
                                         AF.Exp, scale=SCALE)
                    if kt >= qb * 4:  # diagonal straddle: first 128 cols
                        nc.vector.tensor_mul(e[:, 0:128], e[:, 0:128],
                                             tri_sb[:])
                    return (e, off, N)

                def emit_rsav(u, kt, e, off, N):
                    qb, h = units[u]
                    if kt == 0:
                        rs_ps[u] = pp.tile([128, 512], F32, tag="bank",
                                           name="rsps")
                        ctx_ps[u] = pp.tile([128, 512], F32, tag="bank",
                                            name="ctxps")
                    first, last = kt == 0, kt == n_kt[u] - 1
                    nc.tensor.matmul(rs_ps[u][:, off:512], oc_sb[:],
                                     e[:, 0:N], start=first, stop=last)
                    nc.tensor.matmul(ctx_ps[u][:, off:512],
                                     vn[b][:, kt * 128:(kt + 1) * 128],
                                     e[:, 0:N], start=first, stop=last)
                    if last:
                        rcp = sp.tile([128, 512], F32, tag="rcp")
                        nc.vector.reciprocal_approx_fast(
                            out=rcp[:], in_=rs_ps[u][:])
                        ctxt = xcp.tile([128, 512], F16, tag="ctx")
                        nc.vector.tensor_mul(ctxt[:], ctx_ps[u][:], rcp[:])
                        # one DMA writes all 4 dup blocks: src broadcasts
                        # via a stride-0 dim after the partition dim
                        cap = ctxt[:]
                        bsrc = bass.AP(cap.tensor, cap.offset,
                                       [cap.ap[0], (0, 4), cap.ap[1]])
                        dst = a2a_in[b].rearrange("j p t -> p j t")[
                            h * 128:(h + 1) * 128, qb * 4:qb * 4 + 4, :]
                        nc.sync.dma_start(dst, bsrc)

                pend = {}
                mid_at = tasks.index((6, 0)) if mid_hook else -1
                for i, (u, kt) in enumerate(tasks):
                    if i == mid_at:
                        mid_hook()
                    pend[i] = (u, kt) + emit_sc(u, kt)
                    if i - 3 >= 0:
                        emit_rsav(*pend.pop(i - 3))
                for j in sorted(pend):
                    emit_rsav(*pend.pop(j))

            def a2a(b):
                nc.gpsimd.collective_compute(
                    "AllToAll", ALU.bypass,
                    replica_groups=[list(range(N_CORES))],
                    ins=[a2a_in[b][:].opt()],
                    outs=[a2a_out[b][:].opt()])

            load_wg(0)
            load_wg(1)
            proj_block(0)
            proj_block(1)
            kv_allgather(0)
            proj_block(2)
            assemble_kv(0)
            proj_block(3)
            kv_allgather(1)
            es2.close()

            for i in range(4):
                load_wd(i)
            attn_batch(0, mid_hook=lambda: assemble_kv(1))
            a2a(0)
            qt0.close()

            # gathered-context mega-tile: cg[:, kk, 0:512|512:1024],
            # loaded in 8-kk chunks so dense can start on the first chunk
            cgp = es.enter_context(
                tc.tile_pool(name="cg", bufs=1, side="right"))
            op_ = es.enter_context(
                tc.tile_pool(name="dout", bufs=4, side="right"))
            cg = cgp.tile([128, 32, 1024], F16, tag="cg", name="cg")

            attn_batch(1)
            a2a(1)
            for b_ in range(2):
                cgs = a2a_out[b_].rearrange("s (sub p) t -> p (s sub) t",
                                            sub=4)
                for cc in range(4):
                    nc.sync.dma_start(
                        cg[:, cc * 8:(cc + 1) * 8,
                           b_ * 512:(b_ + 1) * 512],
                        cgs[:, cc * 8:(cc + 1) * 8, :])

            # dense: out[od 1024, tok 1024]; all batch-0 half-passes first
            # (55us of work independent of the second AllToAll), then the
            # batch-1 half-passes.
            def dense_half(odb, half):
                ps = pp.tile([128, 512], F32, tag="bank",
                             name=f"dops{odb}_{half}")
                for kk in range(32):
                    nc.tensor.matmul(
                        ps[:],
                        wd_sb[kk // 8][:, kk % 8,
                                       odb * 128:(odb + 1) * 128],
                        cg[:, kk, half * 512:(half + 1) * 512],
                        start=(kk == 0), stop=(kk == 31))
                o = op_.tile([128, 512], F32, tag="osb")
                nc.scalar.copy(o[:], ps[:])
                nc.sync.dma_start(
                    out_d[odb * 128:(odb + 1) * 128,
                          half * 512:(half + 1) * 512], o[:])

            for odb in range(8):
                dense_half(odb, 0)
            for odb in range(8):
                dense_half(odb, 1)
            es.close()

    nc.compile()
    return nc


_CACHE = {}


def _get_nc():
    if "nc" not in _CACHE:
        _CACHE["nc"] = build()
    return _CACHE["nc"]


def _host_prep(hidden_states, rope_cache, w_qkv, b_qkv, w_dense):
    """Build the 8 per-core input maps."""
    x = np.ascontiguousarray(hidden_states.reshape(TOK, H))
    xt = np.ascontiguousarray(x.T).astype(np.float16)

    # rope coefficient planes [64, TOK], token index j = b*S + s
    c0 = np.transpose(rope_cache[:, :, :, 0], (2, 1, 0)).reshape(ROT // 2, TOK)
    c1 = np.transpose(rope_cache[:, :, :, 1], (2, 1, 0)).reshape(ROT // 2, TOK)
    ra = np.repeat(c0, 2, axis=0).astype(np.float32)
    rb = np.repeat(c1, 2, axis=0).astype(np.float32)
    rb[0::2] *= -1.0
    rq = np.ascontiguousarray(np.vstack([ra, rb]))

    perm = np.zeros((ROT, ROT), np.float32)
    for k in range(ROT):
        perm[k, k ^ 1] = 1.0
    cc = np.zeros((128, 448), np.float32)
    cc[:, 0:128] = 1.0                                  # ones
    cc[:, 128:256] = np.triu(np.ones((128, 128)))       # tri[k,q]=1 iff q>=k
    cc[:, 256:384] = np.eye(128)                        # ident
    cc[0:64, 384:448] = perm
    cc = cc.astype(np.float16)

    in_maps = []
    for c in range(N_CORES):
        g = c // (N_CORES // G)     # KV group
        r = c % (N_CORES // G)      # rank within KV group
        oi = c % 4                  # dense output-quarter
        kc0 = NH * HD + g * HD + 32 * r          # K col slice start
        vc0 = NH * HD + G * HD + g * HD + 32 * r  # V col slice start
        wq_c = np.concatenate([
            w_qkv[:, c * DPC:(c + 1) * DPC],
            w_qkv[:, kc0:kc0 + 32],
            w_qkv[:, vc0:vc0 + 32],
        ], axis=1)
        bq_c = np.zeros((128, NDB), np.float32)
        bq_c[:, 0:4] = b_qkv[c * DPC:(c + 1) * DPC].reshape(4, 128).T
        bq_c[0:32, 4] = b_qkv[kc0:kc0 + 32]
        bq_c[32:64, 4] = b_qkv[vc0:vc0 + 32]
        if r < 2:
            rak = ra[32 * r:32 * r + 32]
            rbk = rb[32 * r:32 * r + 32]
        else:  # pass-dims: rope is identity
            rak = np.ones((32, TOK), np.float32)
            rbk = np.zeros((32, TOK), np.float32)
        in_maps.append({
            "xt": xt,
            "wqkv": wq_c.astype(np.float16),
            "bqkv": np.ascontiguousarray(bq_c),
            "consts": cc,
            "ropeQ": rq,
            "ropeK": np.ascontiguousarray(np.vstack([rak, rbk])),
            "wd": w_dense[:, oi * ODPC:(oi + 1) * ODPC].astype(np.float16),
        })
    return in_maps


def kernel(hidden_states, rope_cache, w_qkv, b_qkv, w_dense,
           _trace=False, _trace_cores=None):
    nc = _get_nc()
    in_maps = _host_prep(np.asarray(hidden_states), np.asarray(rope_cache),
                         np.asarray(w_qkv), np.asarray(b_qkv),
                         np.asarray(w_dense))
    res = run_bass_kernel_spmd(nc, in_maps, core_ids=list(range(N_CORES)),
                               trace=_trace, trace_cores=_trace_cores)
    _CACHE["last_result"] = res
    full = np.empty((TOK, H), np.float32)
    for c in range(N_CORES):
        ti, oi = c // 4, c % 4
        o = res.results[c]["out"]                 # [1024 od, 1024 tok]
        for b in range(B):
            full[b * S + ti * 512:b * S + (ti + 1) * 512,
                 oi * ODPC:(oi + 1) * ODPC] = o[:, b * 512:(b + 1) * 512].T
    return full.reshape(B, S, H)


# revision 24
# speedup vs baseline: 1.2657x; 1.1778x over previous
"""ChatGLM2 attention block on 8 Trainium2 NeuronCores (Bass/Tile).

Sharding: tensor-parallel across heads for QKV+attention; each core c owns
Q heads 4c..4c+3 (512 dims). K/V projection is split 4-ways inside each KV
group: core with group-rank r computes a 64-col [K 32 | V 32] slice, and a
small intra-group AllGather (128KB in) rebuilds the full K/V per batch.
Dense is sharded 2x4 (token-half x output-quarter): the per-core context is
redistributed with an AllToAll (stride-0-broadcast input, 2MB staged)
instead of a 16MB AllGather, and each core computes out[1024 od, 1024 tok]
against its w_dense column slice.

DMA triggers are the scarce resource (~0.7us dispatch each, serialized per
engine sequencer): they are batched into few large transfers (packed
consts, one DMA per weight group, packed rope planes, one mega-AP DMA for
the whole gathered-context tile) and spread across the gpsimd / vector /
scalar queues so the sync queue never backs up in front of a collective
trigger or a proj input load.

All matmuls run fp16 (1 cycle/row). Softmax skips the row-max (scores are
~1e-2); the denominator is a ones-matrix matmul accumulated alongside the
AV matmul; causal masking is a DVE multiply with a 128x128 triangular tile
on only the diagonal 128 columns of each straddling k-tile. The attention
inner loop is software-pipelined with lookahead 2 so the PE never waits on
the exp->mask chain.
"""

import math
import sys
import types

import numpy as np

# ---------------------------------------------------------------- constants
B, S, H = 2, 1024, 4096
NH, G, HD = 32, 2, 128
ROT = 64
N_CORES = 8
TOK = B * S                      # 2048
HPC = NH // N_CORES              # 4 Q heads per core
DPC = HPC * HD                   # 512 Q dims per core
NDB = 5                          # per-core qkv dim blocks: 4x Q(128) + KV(64)
WQC = DPC + 64                   # 576 qkv cols per core
TB = 4                           # token blocks of 512
QB = 2                           # q blocks of 512 per batch
ODPC = H // 4                    # 1024 output dims per core (dense quarter)
SCALE = 1.0 / math.sqrt(HD)


def _install_ntff_hook():
    """The agent image's antenv lacks axon_hooks; shim it so
    run_bass_kernel_spmd(trace=True) can profile via NTFF."""
    if "antenv.axon_hooks" in sys.modules:
        return
    import antenv  # noqa: F401

    mod = types.ModuleType("antenv.axon_hooks")
    mod._hook = None
    mod.set_axon_ntff_profile_hook = lambda h: setattr(mod, "_hook", h)
    mod.get_axon_ntff_profile_hook = lambda: mod._hook
    sys.modules["antenv.axon_hooks"] = mod
    try:
        from trn_agent_boot.trn_boot import _ntff_profile_via_ctypes

        mod._hook = _ntff_profile_via_ctypes("/opt/axon/libaxon_pjrt.so")
    except Exception:
        pass


_install_ntff_hook()

import concourse.bass as bass  # noqa: E402
import concourse.mybir as mybir  # noqa: E402
import concourse.tile as tile  # noqa: E402
from concourse import bacc  # noqa: E402
from concourse.bass_utils import run_bass_kernel_spmd  # noqa: E402

F32 = mybir.dt.float32
F16 = mybir.dt.float16
AF = mybir.ActivationFunctionType
ALU = mybir.AluOpType


# ---------------------------------------------------------------- build
def build(trace_label="k"):
    nc = bacc.Bacc("TRN2", target_bir_lowering=False, debug=False,
                   num_devices=N_CORES)

    xt_d = nc.dram_tensor("xt", [H, TOK], F16, kind="ExternalInput").ap()
    wq_d = nc.dram_tensor("wqkv", [H, WQC], F16, kind="ExternalInput").ap()
    bq_d = nc.dram_tensor("bqkv", [128, NDB], F32, kind="ExternalInput").ap()
    # packed f16 consts: [ ones(128) | tri(128) | ident(128) | perm(64c) ]
    cc_d = nc.dram_tensor("consts", [128, 448], F16, kind="ExternalInput").ap()
    # packed rope planes: ropeQ = [ra;rb] (128 rows), ropeK = [rak;rbk] (64)
    rq_d = nc.dram_tensor("ropeQ", [128, TOK], F32, kind="ExternalInput").ap()
    rk_d = nc.dram_tensor("ropeK", [64, TOK], F32, kind="ExternalInput").ap()
    wd_d = nc.dram_tensor("wd", [H, ODPC], F16, kind="ExternalInput").ap()
    out_d = nc.dram_tensor("out", [ODPC, 1024], F32, kind="ExternalOutput").ap()

    from contextlib import ExitStack

    with tile.TileContext(nc) as tc:
        with tc.tile_pool(name="consts", bufs=1) as cp, \
             tc.tile_pool(name="dram", bufs=1, space="DRAM") as dp:
            # ---- small constants (alive whole kernel)
            bias_sb = cp.tile([128, NDB], F32, tag="bias")
            cc_sb = cp.tile([128, 448], F16, tag="consts")
            nc.sync.dma_start(bias_sb[:], bq_d[:])
            nc.sync.dma_start(cc_sb[:], cc_d[:])
            oc_sb = cc_sb[:, 0:128]
            tri_sb = cc_sb[:, 128:256]
            id_sb = cc_sb[:, 256:384]
            perm_sb = cc_sb[0:64, 384:448]

            # DRAM staging for collectives
            kv_loc = [dp.tile([64, 1024], F16, tag=f"kvl{b}",
                              name=f"kvl{b}") for b in range(B)]
            kvg = [dp.tile([4, 64, 1024], F16, tag=f"kvg{b}",
                           name=f"kvg{b}") for b in range(B)]
            a2a_in = [dp.tile([8, 512, 512], F16, tag=f"a2i{b}",
                              name=f"a2i{b}") for b in range(B)]
            a2a_out = [dp.tile([8, 512, 512], F16, tag=f"a2o{b}",
                               name=f"a2o{b}") for b in range(B)]

            wq_r = wq_d.rearrange("(k p) d -> p k d", p=128)
            xt_r = xt_d.rearrange("(k p) t -> p k t", p=128)
            wd_r = wd_d.rearrange("(k p) n -> p k n", p=128)

            es = ExitStack()
            pp = es.enter_context(
                tc.tile_pool(name="ps_main", bufs=8, space="PSUM"))
            kp = es.enter_context(tc.tile_pool(name="kvp", bufs=1))
            qtp1 = es.enter_context(tc.tile_pool(name="qtp1", bufs=1))
            xcp = es.enter_context(tc.tile_pool(name="ctxp", bufs=5))
            ep = es.enter_context(tc.tile_pool(name="exp", bufs=8))
            sp = es.enter_context(tc.tile_pool(name="att_small", bufs=2))
            wdp = es.enter_context(
                tc.tile_pool(name="wd", bufs=1, side="right"))
            qt0 = ExitStack()
            qtp0 = qt0.enter_context(tc.tile_pool(name="qtp0", bufs=1))
            es2 = ExitStack()
            wp = es2.enter_context(tc.tile_pool(name="wq", bufs=1))
            xp = es2.enter_context(tc.tile_pool(name="xt", bufs=5))
            rp = es2.enter_context(tc.tile_pool(name="rope_tmp", bufs=2))
            abp = es2.enter_context(tc.tile_pool(name="ropeab", bufs=2))
            kqp = es2.enter_context(tc.tile_pool(name="kq", bufs=2))

            # per-batch K / V(transposed) tiles
            ktile = [kp.tile([128, 1024], F16, tag=f"k{b}", name=f"k{b}")
                     for b in range(B)]
            vtile = [kp.tile([128, 1024], F16, tag=f"vt{b}", name=f"vt{b}")
                     for b in range(B)]
            vn = [kp.tile([128, 1024], F16, tag=f"vn{b}", name=f"vn{b}")
                  for b in range(B)]
            qtl = {}
            w_sb = [None] * 8
            wd_sb = []

            def load_wg(g):
                if w_sb[g] is None:
                    wg = wp.tile([128, 4, WQC], F16,
                                 tag=f"wq{g}", name=f"wq{g}")
                    nc.sync.dma_start(wg[:], wq_r[:, g * 4:(g + 1) * 4, :])
                    w_sb[g] = wg

            def load_wd(i):
                wg = wdp.tile([128, 8, ODPC], F16, tag=f"wd{i}",
                              name=f"wdg{i}")
                nc.sync.dma_start(wg[:], wd_r[:, i * 8:(i + 1) * 8, :])
                wd_sb.append(wg)

            def proj_block(t):
                """QKV^T projection + bias + RoPE for one 512-token block.
                Q: 4x128 dims; KV: 64 dims ([K32|V32], rank slice)."""
                b, half = t // 2, t % 2
                ps = [pp.tile([128, 512], F32, tag="bank",
                              name=f"qkvps{d}") for d in range(NDB)]
                xgs = []
                for gg in range(4):
                    xg = xp.tile([128, 8, 512], F16, tag="xtblk")
                    xgs.append(xg)
                    nc.sync.dma_start(
                        xg[:], xt_r[:, gg * 8:(gg + 1) * 8,
                                    t * 512:(t + 1) * 512])
                    if t == 0 and gg < 3:
                        load_wg(gg * 2 + 2)
                        load_wg(gg * 2 + 3)
                    for kk in range(8):
                        g, k = (gg * 8 + kk) // 4, kk % 4
                        for d in range(4):
                            nc.tensor.matmul(
                                ps[d][:],
                                w_sb[g][:, k, d * 128:(d + 1) * 128],
                                xg[:, kk, :],
                                start=(gg == 0 and kk == 0),
                                stop=(gg == 3 and kk == 7),
                            )
                # all 64-wide KV matmuls back-to-back: avoids a PE
                # tile-mode switch on every 5th matmul
                for gg in range(4):
                    for kk in range(8):
                        g, k = (gg * 8 + kk) // 4, kk % 4
                        nc.tensor.matmul(
                            ps[4][0:64, :],
                            w_sb[g][:, k, 512:576],
                            xgs[gg][:, kk, :],
                            start=(gg == 0 and kk == 0),
                            stop=(gg == 3 and kk == 7),
                        )
                for h in range(HPC):
                    qt = (qtp0 if t < 2 else qtp1).tile(
                        [128, 512], F16, tag=f"q{h}_{t}", name=f"q{h}_{t}")
                    qtl[(h, t)] = qt
                    nc.scalar.activation(qt[:], ps[h][:], AF.Identity,
                                         bias=bias_sb[:, h:h + 1])
                kq = kqp.tile([64, 512], F16, tag="kvtile", name=f"kv{t}")
                nc.scalar.activation(kq[:], ps[4][0:64, :], AF.Identity,
                                     bias=bias_sb[0:64, 4:5])
                tsl = slice(t * 512, (t + 1) * 512)
                qab = abp.tile([128, 512], F32, tag="ropeQb")
                nc.sync.dma_start(qab[:], rq_d[:, tsl])
                kab = abp.tile([64, 512], F32, tag="ropeKb", bufs=1)
                nc.sync.dma_start(kab[:], rk_d[:, tsl])
                for e in range(HPC):
                    qt = qtl[(e, t)]
                    sw = pp.tile([128, 512], F32, tag="bank", name="swps")
                    nc.tensor.matmul(sw[0:ROT, :], perm_sb[:],
                                     qt[0:ROT, :], start=True, stop=True)
                    t1 = rp.tile([ROT, 512], F32, tag="t1")
                    nc.vector.tensor_mul(t1[:], qt[0:ROT, :], qab[0:64, :])
                    t2 = rp.tile([ROT, 512], F32, tag="t2")
                    nc.vector.tensor_mul(t2[:], sw[0:ROT, :], qab[64:128, :])
                    nc.vector.tensor_add(qt[0:ROT, :], t1[:], t2[:])
                # K slice rope (identity planes on ranks holding pass-dims)
                swk = pp.tile([128, 512], F32, tag="bank", name="swkps")
                nc.tensor.matmul(swk[0:32, :], perm_sb[0:32, 0:32],
                                 kq[0:32, :], start=True, stop=True)
                t1 = rp.tile([32, 512], F32, tag="t1k", bufs=1)
                nc.vector.tensor_mul(t1[:], kq[0:32, :], kab[0:32, :])
                t2 = rp.tile([32, 512], F32, tag="t2k", bufs=1)
                nc.vector.tensor_mul(t2[:], swk[0:32, :], kab[32:64, :])
                nc.vector.tensor_add(kq[0:32, :], t1[:], t2[:])
                nc.scalar.dma_start(
                    kv_loc[b][:, half * 512:(half + 1) * 512], kq[:])

            def kv_allgather(b):
                nc.gpsimd.collective_compute(
                    "AllGather", ALU.bypass,
                    replica_groups=[[0, 1, 2, 3], [4, 5, 6, 7]],
                    ins=[kv_loc[b][:].opt()],
                    outs=[kvg[b][:].opt()])

            def assemble_kv(b):
                """Rebuild K [128,1024] and V-transposed [tok,HD] from the
                gathered per-rank 64-col slices."""
                for r in range(4):
                    nc.gpsimd.dma_start(ktile[b][32 * r:32 * r + 32, :],
                                        kvg[b][r, 0:32, :])
                    nc.gpsimd.dma_start(vtile[b][32 * r:32 * r + 32, :],
                                        kvg[b][r, 32:64, :])
                for jj in range(2):
                    tp = pp.tile([128, 512], F16, tag="bank", name="vtrps")
                    for j in range(4):
                        nc.tensor.transpose(
                            tp[:, j * 128:(j + 1) * 128],
                            vtile[b][:, (jj * 4 + j) * 128:
                                     (jj * 4 + j + 1) * 128],
                            id_sb[:])
                    nc.scalar.copy(vn[b][:, jj * 512:(jj + 1) * 512], tp[:])

            def attn_batch(b, mid_hook=None):
                """Software-pipelined attention for one batch: flattened
                (qb, h, kt) stream with lookahead-2 sc -> rs/av."""
                units = [(qb, h) for qb in range(QB) for h in range(HPC)]
                tasks = []
                for u, (qb, h) in enumerate(units):
                    for kt in range(4 * (qb + 1)):
                        tasks.append((u, kt))
                n_kt = {u: 4 * (units[u][0] + 1) for u in range(len(units))}
                rs_ps, ctx_ps = {}, {}

                def emit_sc(u, kt):
                    qb, h = units[u]
                    tb = b * QB + qb
                    off = max(0, (kt - qb * 4) * 128)
                    N = 512 - off
                    sc = pp.tile([128, 512], F32, tag="bank", name="scps")
                    nc.tensor.matmul(sc[:, 0:N],
                                     ktile[b][:, kt * 128:(kt + 1) * 128],
                                     qtl[(h, tb)][:, off:512],
                                     start=True, stop=True)
                    e = ep.tile([128, 512], F16, tag="exp")
                    nc.scalar.activation(e[:, 0:N], sc[:, 0:N],
                                         AF.Exp, scale=SCALE)
                    if kt >= qb * 4:  # diagonal straddle: first 128 cols
                        nc.vector.tensor_mul(e[:, 0:128], e[:, 0:128],
                                             tri_sb[:])
                    return (e, off, N)

                def emit_rsav(u, kt, e, off, N):
                    qb, h = units[u]
                    if kt == 0:
                        rs_ps[u] = pp.tile([128, 512], F32, tag="bank",
                                           name="rsps")
                        ctx_ps[u] = pp.tile([128, 512], F32, tag="bank",
                                            name="ctxps")
                    first, last = kt == 0, kt == n_kt[u] - 1
                    nc.tensor.matmul(rs_ps[u][:, off:512], oc_sb[:],
                                     e[:, 0:N], start=first, stop=last)
                    nc.tensor.matmul(ctx_ps[u][:, off:512],
                                     vn[b][:, kt * 128:(kt + 1) * 128],
                                     e[:, 0:N], start=first, stop=last)
                    if last:
                        rcp = sp.tile([128, 512], F32, tag="rcp")
                        nc.vector.reciprocal_approx_fast(
                            out=rcp[:], in_=rs_ps[u][:])
                        ctxt = xcp.tile([128, 512], F16, tag="ctx")
                        nc.vector.tensor_mul(ctxt[:], ctx_ps[u][:], rcp[:])
                        # one DMA writes all 4 dup blocks: src broadcasts
                        # via a stride-0 dim after the partition dim
                        cap = ctxt[:]
                        bsrc = bass.AP(cap.tensor, cap.offset,
                                       [cap.ap[0], (0, 4), cap.ap[1]])
                        dst = a2a_in[b].rearrange("j p t -> p j t")[
                            h * 128:(h + 1) * 128, qb * 4:qb * 4 + 4, :]
                        nc.sync.dma_start(dst, bsrc)

                pend = {}
                for i, (u, kt) in enumerate(tasks):
                    pend[i] = (u, kt) + emit_sc(u, kt)
                    if i - 3 >= 0:
                        emit_rsav(*pend.pop(i - 3))
                for j in sorted(pend):
                    emit_rsav(*pend.pop(j))
                if mid_hook:
                    mid_hook()

            def a2a(b):
                nc.gpsimd.collective_compute(
                    "AllToAll", ALU.bypass,
                    replica_groups=[list(range(N_CORES))],
                    ins=[a2a_in[b][:].opt()],
                    outs=[a2a_out[b][:].opt()])

            load_wg(0)
            load_wg(1)
            proj_block(0)
            proj_block(1)
            kv_allgather(0)
            proj_block(2)
            assemble_kv(0)
            proj_block(3)
            kv_allgather(1)
            es2.close()

            for i in range(4):
                load_wd(i)
            attn_batch(0, mid_hook=lambda: assemble_kv(1))
            a2a(0)
            qt0.close()

            # gathered-context mega-tile: cg[:, kk, 0:512|512:1024],
            # loaded in 8-kk chunks so dense can start on the first chunk
            cgp = es.enter_context(
                tc.tile_pool(name="cg", bufs=1, side="right"))
            op_ = es.enter_context(
                tc.tile_pool(name="dout", bufs=4, side="right"))
            cg = [cgp.tile([128, 32, 512], F16, tag=f"cg{b_}",
                           name=f"cg{b_}") for b_ in range(2)]

            attn_batch(1)
            a2a(1)
            for b_ in range(2):
                cgs = a2a_out[b_].rearrange("s (sub p) t -> p (s sub) t",
                                            sub=4)
                for cc in range(4):
                    nc.sync.dma_start(
                        cg[b_][:, cc * 8:(cc + 1) * 8, :],
                        cgs[:, cc * 8:(cc + 1) * 8, :])

            # dense: out[od 1024, tok 1024]; all batch-0 half-passes first
            # (55us of work independent of the second AllToAll), then the
            # batch-1 half-passes.
            def dense_half(odb, half):
                ps = pp.tile([128, 512], F32, tag="bank",
                             name=f"dops{odb}_{half}")
                for kk in range(32):
                    nc.tensor.matmul(
                        ps[:],
                        wd_sb[kk // 8][:, kk % 8,
                                       odb * 128:(odb + 1) * 128],
                        cg[half][:, kk, :],
                        start=(kk == 0), stop=(kk == 31))
                o = op_.tile([128, 512], F32, tag="osb")
                nc.scalar.copy(o[:], ps[:])
                nc.sync.dma_start(
                    out_d[odb * 128:(odb + 1) * 128,
                          half * 512:(half + 1) * 512], o[:])

            for odb in range(8):
                dense_half(odb, 0)
            for odb in range(8):
                dense_half(odb, 1)
            es.close()

    nc.compile()
    return nc


_CACHE = {}


def _get_nc():
    if "nc" not in _CACHE:
        _CACHE["nc"] = build()
    return _CACHE["nc"]


def _host_prep(hidden_states, rope_cache, w_qkv, b_qkv, w_dense):
    """Build the 8 per-core input maps."""
    x = np.ascontiguousarray(hidden_states.reshape(TOK, H))
    xt = np.ascontiguousarray(x.T).astype(np.float16)

    # rope coefficient planes [64, TOK], token index j = b*S + s
    c0 = np.transpose(rope_cache[:, :, :, 0], (2, 1, 0)).reshape(ROT // 2, TOK)
    c1 = np.transpose(rope_cache[:, :, :, 1], (2, 1, 0)).reshape(ROT // 2, TOK)
    ra = np.repeat(c0, 2, axis=0).astype(np.float32)
    rb = np.repeat(c1, 2, axis=0).astype(np.float32)
    rb[0::2] *= -1.0
    rq = np.ascontiguousarray(np.vstack([ra, rb]))

    perm = np.zeros((ROT, ROT), np.float32)
    for k in range(ROT):
        perm[k, k ^ 1] = 1.0
    cc = np.zeros((128, 448), np.float32)
    cc[:, 0:128] = 1.0                                  # ones
    cc[:, 128:256] = np.triu(np.ones((128, 128)))       # tri[k,q]=1 iff q>=k
    cc[:, 256:384] = np.eye(128)                        # ident
    cc[0:64, 384:448] = perm
    cc = cc.astype(np.float16)

    in_maps = []
    for c in range(N_CORES):
        g = c // (N_CORES // G)     # KV group
        r = c % (N_CORES // G)      # rank within KV group
        oi = c % 4                  # dense output-quarter
        kc0 = NH * HD + g * HD + 32 * r          # K col slice start
        vc0 = NH * HD + G * HD + g * HD + 32 * r  # V col slice start
        wq_c = np.concatenate([
            w_qkv[:, c * DPC:(c + 1) * DPC],
            w_qkv[:, kc0:kc0 + 32],
            w_qkv[:, vc0:vc0 + 32],
        ], axis=1)
        bq_c = np.zeros((128, NDB), np.float32)
        bq_c[:, 0:4] = b_qkv[c * DPC:(c + 1) * DPC].reshape(4, 128).T
        bq_c[0:32, 4] = b_qkv[kc0:kc0 + 32]
        bq_c[32:64, 4] = b_qkv[vc0:vc0 + 32]
        if r < 2:
            rak = ra[32 * r:32 * r + 32]
            rbk = rb[32 * r:32 * r + 32]
        else:  # pass-dims: rope is identity
            rak = np.ones((32, TOK), np.float32)
            rbk = np.zeros((32, TOK), np.float32)
        in_maps.append({
            "xt": xt,
            "wqkv": wq_c.astype(np.float16),
            "bqkv": np.ascontiguousarray(bq_c),
            "consts": cc,
            "ropeQ": rq,
            "ropeK": np.ascontiguousarray(np.vstack([rak, rbk])),
            "wd": w_dense[:, oi * ODPC:(oi + 1) * ODPC].astype(np.float16),
        })
    return in_maps


def kernel(hidden_states, rope_cache, w_qkv, b_qkv, w_dense,
           _trace=False, _trace_cores=None):
    nc = _get_nc()
    in_maps = _host_prep(np.asarray(hidden_states), np.asarray(rope_cache),
                         np.asarray(w_qkv), np.asarray(b_qkv),
                         np.asarray(w_dense))
    res = run_bass_kernel_spmd(nc, in_maps, core_ids=list(range(N_CORES)),
                               trace=_trace, trace_cores=_trace_cores)
    _CACHE["last_result"] = res
    full = np.empty((TOK, H), np.float32)
    for c in range(N_CORES):
        ti, oi = c // 4, c % 4
        o = res.results[c]["out"]                 # [1024 od, 1024 tok]
        for b in range(B):
            full[b * S + ti * 512:b * S + (ti + 1) * 512,
                 oi * ODPC:(oi + 1) * ODPC] = o[:, b * 512:(b + 1) * 512].T
    return full.reshape(B, S, H)
